# revision 3
# baseline (speedup 1.0000x reference)
"""Trainium2 Bass kernel for per-sample reflect-pad + random-crop +
brightness/contrast jitter + quantize (DRAC transform).

Design notes (found via TimelineSim + HW micro-benchmarks):
- Gather: ONE indirect-DMA descriptor per sample (a contiguous 14700B
  slab covering all 3 channel crops), 128 per chunk, 256 per core.
  Real SWDGE desc-gen costs ~30ns/descriptor (4x the cost model), so
  descriptor count matters more than fine-grained gather splits; the
  v1 baseline's per-(sample,channel)-half gather used 1536.
- Compute: per (chunk, channel) path config. P1: ACT does the spatial
  sum via activation(Identity, accum_out=...) whose main output doubles
  as a CONTIGUOUS u8 copy of the strided crop, then DVE computes
  z = x*f + b from that copy at full rate (2.2us vs 4.3us strided).
  P3 (one channel per program): DVE strided sum + strided z, which
  offloads the ACT critical chain. Sums/z interleave so both engines
  stay busy; per-channel stores drain as soon as each z finishes.
- Emission order g0,s0,g1,t0,s1,t1 keeps both engine queues free of
  head-of-line blocking on the later chunk's data.

Math (255-scaled space; output convert saturates+rounds to u8):
  f = 0.1*jc + 0.95
  d = 25.5*jb - 12.75
  b_c = sum_c * (1-f)/4096 + d
  z = clip(round(x*f + b_c), 0, 255)
"""
import numpy as np

PAD = 3
B, C, H, W = 2048, 3, 64, 64
HP, WP = H + 2 * PAD, W + 2 * PAD          # 70, 70
NCORES = 8
BS = B // NCORES                            # 256 samples per core
SROW = C * HP * WP                          # 14700 elements per padded sample
CSTR = HP * WP                              # 4900 per channel
PX = H * W                                  # 4096
OUTW = C * PX                               # 12288
CHP = 128                                   # samples per chunk (partition dim)
NCHUNK = BS // CHP                          # 2

# engine path per (chunk, channel): "P1" ACT sum -> DVE z (contiguous),
# "P3" DVE strided sum -> DVE strided z, "P2" DVE strided sum -> ACT z.
PATH = {(0, 0): "P3", (0, 1): "P1", (0, 2): "P1",
        (1, 0): "P1", (1, 1): "P2", (1, 2): "P1"}

_prog = None
TRACE = False
LAST_RESULT = None


def _build_program():
    from contextlib import ExitStack
    from concourse import bass, bacc, mybir, tile

    f32, i32, u8 = mybir.dt.float32, mybir.dt.int32, mybir.dt.uint8
    AF = mybir.ActivationFunctionType
    OP = mybir.AluOpType
    AX = mybir.AxisListType

    nc = bacc.Bacc("TRN2", target_bir_lowering=False, debug=False)
    xp = nc.dram_tensor("xp", [1, BS * SROW + 1024], u8, kind="ExternalInput")
    idx = nc.dram_tensor("idx", [CHP, NCHUNK], i32, kind="ExternalInput")
    jbr = nc.dram_tensor("jbr", [CHP, NCHUNK], f32, kind="ExternalInput")
    jcr = nc.dram_tensor("jcr", [CHP, NCHUNK], f32, kind="ExternalInput")
    out = nc.dram_tensor("out", [BS, OUTW], u8, kind="ExternalOutput")

    with tile.TileContext(nc) as tc, ExitStack() as ctx:
        const = ctx.enter_context(tc.tile_pool(name="const", bufs=1))
        idx_t = const.tile([CHP, NCHUNK], i32)
        nc.sync.dma_start(idx_t[:], idx[:, :])

        # dep-free dummy activation hoists the ACT function-table load
        warm = const.tile([1, 1], f32)
        nc.gpsimd.memset(warm[:], 0.0)
        nc.scalar.activation(warm[:], warm[:], AF.Identity)

        jb_t = const.tile([CHP, NCHUNK], f32)
        nc.sync.dma_start(jb_t[:], jbr[:, :])
        jc_t = const.tile([CHP, NCHUNK], f32)
        nc.sync.dma_start(jc_t[:], jcr[:, :])

        xpool = ctx.enter_context(tc.tile_pool(name="x", bufs=2))
        dpool = ctx.enter_context(tc.tile_pool(name="dmp", bufs=2))
        opool = ctx.enter_context(tc.tile_pool(name="o", bufs=2))
        tpool = ctx.enter_context(tc.tile_pool(name="t", bufs=NCHUNK))

        scal = []
        for ci in range(NCHUNK):
            fT = tpool.tile([CHP, 1], f32, tag="f")
            nc.vector.tensor_scalar(fT[:], jc_t[:, ci:ci + 1], 0.1, 0.95,
                                    OP.mult, OP.add)
            dT = tpool.tile([CHP, 1], f32, tag="d")
            nc.vector.tensor_scalar(dT[:], jb_t[:, ci:ci + 1], 25.5, -12.75,
                                    OP.mult, OP.add)
            o4 = tpool.tile([CHP, 1], f32, tag="o4")
            nc.vector.tensor_scalar(o4[:], fT[:], -1.0 / PX, 1.0 / PX,
                                    OP.mult, OP.add)
            scal.append((fT, dT, o4))

        def crop3d(tl, base):
            v = tl[:, base:base + H * WP]
            return v.rearrange("p (h w) -> p h w", h=H, w=WP)[:, :, :W]

        def emit_gather(ci):
            # fetch only the span the 3 channel views read (2*CSTR + H*WP =
            # 14280B); the <=426B overhang past a sample's row is harmless
            # read-only overfetch into the next sample / the xp tail pad
            slab = xpool.tile([CHP, 2 * CSTR + H * WP], u8, tag="slab")
            nc.gpsimd.indirect_dma_start(
                out=slab[:], out_offset=None, in_=xp[:, :],
                in_offset=bass.IndirectOffsetOnAxis(
                    ap=idx_t[:, ci:ci + 1], axis=1))
            return [crop3d(slab, c * CSTR) for c in range(C)]

        def emit_sums(ci, crops):
            ssum = tpool.tile([CHP, C], f32, tag="ssum")
            dumps = [None] * C
            for c in range(C):
                if PATH[(ci, c)] == "P1":
                    dump = dpool.tile([CHP, PX], u8, tag=f"dump{c}")
                    d3 = dump.rearrange("p (h w) -> p h w", h=H, w=W)
                    nc.scalar.activation(d3, crops[c], AF.Identity,
                                         accum_out=ssum[:, c:c + 1])
                    dumps[c] = dump
                else:
                    nc.vector.tensor_reduce(ssum[:, c:c + 1], crops[c],
                                            AX.XY, OP.add)
            return ssum, dumps

        def emit_tail(ci, crops, ssum, dumps):
            fT, dT, o4 = scal[ci]
            bT = tpool.tile([CHP, C], f32, tag="b")
            zu = opool.tile([CHP, OUTW], u8, tag="zu")
            rows = slice(CHP * ci, CHP * (ci + 1))
            for c in range(C):
                nc.vector.scalar_tensor_tensor(bT[:, c:c + 1], ssum[:, c:c + 1],
                                               o4[:], dT[:], OP.mult, OP.add)
                zslice = zu[:, c * PX:(c + 1) * PX]
                if PATH[(ci, c)] == "P1":
                    nc.vector.tensor_scalar(zslice, dumps[c][:, :],
                                            fT[:], bT[:, c:c + 1],
                                            OP.mult, OP.add)
                elif PATH[(ci, c)] == "P3":
                    z3 = zslice.rearrange("p (h w) -> p h w", h=H, w=W)
                    nc.vector.tensor_scalar(z3, crops[c], fT[:],
                                            bT[:, c:c + 1], OP.mult, OP.add)
                else:  # P2
                    z3 = zslice.rearrange("p (h w) -> p h w", h=H, w=W)
                    nc.scalar.activation(z3, crops[c], AF.Identity,
                                         bias=bT[:, c:c + 1], scale=fT[:])
                nc.sync.dma_start(out[rows, c * PX:(c + 1) * PX], zslice)

        # g0, s0, g1, t0, s1, t1: neither engine queue blocks on later data
        crops0 = emit_gather(0)
        s0 = emit_sums(0, crops0)
        crops1 = emit_gather(1)
        emit_tail(0, crops0, *s0)
        s1 = emit_sums(1, crops1)
        emit_tail(1, crops1, *s1)

    nc.compile()
    return nc


def _host_prep(x_uint8, offs_h, offs_w, jitter_b, jitter_c):
    """Shard + build per-core input maps (padding, dtype repack, and index
    arithmetic only - no image math)."""
    xpad = np.pad(np.asarray(x_uint8).astype(np.uint8),
                  ((0, 0), (0, 0), (PAD, PAD), (PAD, PAD)), mode="reflect")
    oh = np.asarray(offs_h).astype(np.int64).reshape(B)
    ow = np.asarray(offs_w).astype(np.int64).reshape(B)
    jb = np.asarray(jitter_b, dtype=np.float32).reshape(B)
    jc = np.asarray(jitter_c, dtype=np.float32).reshape(B)

    in_maps = []
    for k in range(NCORES):
        sl = slice(k * BS, (k + 1) * BS)
        start = (np.arange(BS, dtype=np.int64) * SROW
                 + oh[sl] * WP + ow[sl])                    # [BS] elem offsets
        idxm = start.reshape(NCHUNK, CHP).T.astype(np.int32).copy()
        jbrm = jb[sl].reshape(NCHUNK, CHP).T.copy()
        jcrm = jc[sl].reshape(NCHUNK, CHP).T.copy()
        xpf = np.zeros((1, BS * SROW + 1024), np.uint8)
        xpf[0, :BS * SROW] = xpad[sl].reshape(-1)
        in_maps.append({"xp": xpf, "idx": idxm, "jbr": jbrm, "jcr": jcrm})
    return in_maps


def kernel(x_uint8, offs_h, offs_w, jitter_b, jitter_c):
    global _prog, LAST_RESULT
    from concourse.bass_utils import run_bass_kernel_spmd

    if _prog is None:
        _prog = _build_program()

    in_maps = _host_prep(x_uint8, offs_h, offs_w, jitter_b, jitter_c)
    res = run_bass_kernel_spmd(_prog, in_maps, list(range(NCORES)), trace=TRACE)
    LAST_RESULT = res
    outs = [res.results[k]["out"].reshape(BS, C, H, W) for k in range(NCORES)]
    return np.concatenate(outs, axis=0).astype(np.int32)  # lossless: values in [0,255]
